# revision 7
# baseline (speedup 1.0000x reference)
"""ApproxCompressor Trainium2 kernel (8 NeuronCores, data parallel over batch).

Algorithm: the reference's FFT convolution with the truncated exponential
impulse response h[n] = (1-a) a^n is a one-pole IIR y[t] = a y[t-1] + (1-a) e[t]
minus a tail term a^16384 y[t-16384] that underflows to zero in float32 for
any alpha = sigmoid(randn).  On-device we therefore run an exact recursive
scan instead of an FFT.

Per core: 4 examples.  Each example's L=131072 samples are laid out as
[128 partitions x 1024]; every DMA is a fully contiguous 512KB HBM transfer
and all 128 partitions scan in parallel (DVE tensor_tensor_scan along the
free dim; the scan state is fp32 internally regardless of output dtype).

The whole elementwise path runs in fp16: x is cast f32->fp16 during the
input DMA (SWDGE cast), which makes every DVE tensor_tensor run in 2x mode
and every tensor_scalar in 4x mode; outputs are cast fp16->f32 during the
output DMA.  The energy scale (1-a)/2 is folded into the Ln scale so the
squares need no scaling.

Cross-chunk scan carries are fixed post-hoc: carry[p] (the true initial
state of chunk p) is linear in the per-chunk final values S, carry = M @ S
with M precomputed on host in f64.  TensorE computes carryT = S^T @ M^T
([1,128]) then pc = carry x decay (rank-1) PLUS an identity-matmul
accumulation of y's first nb*512 columns into the same PSUM tile, so the
corrected envelope for those columns materializes in PSUM with no DVE add;
Ln reads it straight from PSUM (ACT is the engine closest to PSUM).

The quadratic-knee gain is refactored into per-partition-scalar ops:
    d    = ln(lnscale * y + lnbias)            (ACT, scale/bias fold)
    u    = clamp(d, -W, W)                     (DVE tensor_scalar 4x)
    t1   = -c * max(d, W)                      (DVE tensor_scalar 4x)
    sqv  = (s*u + s*W)^2,  s = sqrt(-c/(4W))   (ACT square)
    comb = t1 + sqv                            (DVE tensor_tensor 2x)
    gain = exp(-comb - c*W)                    (ACT, bias fold)
which equals exp(c*q(d)) of the reference knee exactly.  out_c = gain * x_c
is written into the (dead) square tiles and DMA'd out with an fp16->f32 cast.

Emission is software-pipelined: the per-example stage chains are interleaved
with a 2-stage offset so each engine's in-order queue never serializes one
example's tail against the next example's head.
"""

import numpy as np

N, C, L = 32, 2, 131072
NCORES = 8
NE = N // NCORES          # examples per core
P = 128                   # partitions = chunks per example
F = L // P                # 1024 samples per partition
BANK = 512                # psum bank width for the carry fix
EPS = 1e-5
OFF = 1                   # pipeline stage offset between examples
NHW = 2                   # examples whose x loads via HWDGE f32 (faster ramp)

_CACHE = {}


def _build(nb):
    import concourse.bass as bass
    import concourse.tile as tile
    from concourse import bacc, mybir

    f32 = mybir.dt.float32
    f16 = mybir.dt.float16
    AF = mybir.ActivationFunctionType
    OP = mybir.AluOpType

    nc = bacc.Bacc("TRN2", target_bir_lowering=False, debug=False, num_devices=NCORES)

    x_h = nc.declare_dram_parameter("x", [NE, C, L], f32, isOutput=False)
    scal_h = nc.declare_dram_parameter("scal", [P, 16 * NE], f32, isOutput=False)
    cmb_h = nc.declare_dram_parameter("cmb", [P, (NE + 1) * P], f16, isOutput=False)
    dec_h = nc.declare_dram_parameter("decay", [1, NE * nb * BANK], f16, isOutput=False)
    out_h = nc.declare_dram_parameter("out", [NE, C, L], f32, isOutput=True)

    from contextlib import ExitStack

    with tile.TileContext(nc) as tc, ExitStack() as ctx:
        const = ctx.enter_context(tc.tile_pool(name="const", bufs=1))
        work = ctx.enter_context(tc.tile_pool(name="work", bufs=4))
        xpool = ctx.enter_context(tc.tile_pool(name="xpool", bufs=4))
        psum = ctx.enter_context(tc.tile_pool(name="psum", bufs=2, space="PSUM"))

        # scal leads (its per-partition scalar columns gate every ACT op),
        # then the x casting loads in pipeline order, then carry constants
        # (not needed until the first carry matmul)
        scal_t = const.tile([P, 16 * NE], f32)
        nc.sync.dma_start(scal_t[:], scal_h[:])

        # tiny dummy activation: hoists the ACT table load off the critical
        # path (otherwise it fires only after the first x DMA lands)
        warm_t = const.tile([P, 1], f32)
        nc.scalar.activation(warm_t[:], scal_t[:, 0:1], AF.Exp, bias=0.0, scale=0.0)

        # first NHW examples load f32 via HWDGE (no Q7 startup, ~2us earlier);
        # their fp16 mul operands are cast on the otherwise-idle GpSimd later.
        # The rest cast f32->fp16 during the SWDGE DMA itself.
        xs = []      # fp16 tiles (mul operands)
        xf = []      # f32 tiles for the HWDGE examples (square operands)
        for e in range(NE):
            x0 = xpool.tile([P, F], f16, tag="x0", name=f"x0e{e}")
            x1 = xpool.tile([P, F], f16, tag="x1", name=f"x1e{e}")
            if e < NHW:
                w0 = xpool.tile([P, F], f32, tag="xf0", name=f"xf0e{e}")
                w1 = xpool.tile([P, F], f32, tag="xf1", name=f"xf1e{e}")
                nc.sync.dma_start(w0[:], x_h[:][e, 0].rearrange("(p i) -> p i", p=P))
                nc.scalar.dma_start(w1[:], x_h[:][e, 1].rearrange("(p i) -> p i", p=P))
                xf.append((w0, w1))
            else:
                nc.gpsimd.dma_start(x0[:], x_h[:][e, 0].rearrange("(p i) -> p i", p=P))
                nc.gpsimd.dma_start(x1[:], x_h[:][e, 1].rearrange("(p i) -> p i", p=P))
            xs.append((x0, x1))
        # GpSimd queue: after the SWDGE in-DMA emissions, cast the HWDGE
        # examples' f32 tiles to fp16 (needed only by the muls, ~20us in)
        for e in range(NHW):
            nc.gpsimd.tensor_copy(xs[e][0][:], xf[e][0][:])
            nc.gpsimd.tensor_copy(xs[e][1][:], xf[e][1][:])
        cmb_t = const.tile([P, (NE + 1) * P], f16)
        nc.sync.dma_start(cmb_t[:], cmb_h[:])
        dec_t = const.tile([1, NE * nb * BANK], f16, padded_shape=[P, NE * nb * BANK])
        nc.sync.dma_start(dec_t[:], dec_h[:])

        def sc(e, j):
            return scal_t[:, 16 * e + j : 16 * e + j + 1]

        def mmt(e):
            return cmb_t[:, e * P : (e + 1) * P]

        ident = cmb_t[:, NE * P : (NE + 1) * P]
        fx = nb * BANK

        # per-example tiles, allocated up front so stages can close over them
        st = []
        for e in range(NE):
            d = {}
            d["sq0"] = work.tile([P, F], f16, tag="sq0", name=f"sq0e{e}")
            d["sq1"] = work.tile([P, F], f16, tag="sq1", name=f"sq1e{e}")
            d["e"] = work.tile([P, F], f16, tag="e", name=f"ee{e}")
            d["y"] = work.tile([P, F], f16, tag="y", name=f"ye{e}")
            d["d"] = work.tile([P, F], f16, tag="d", name=f"de{e}")
            d["u"] = work.tile([P, F], f16, tag="u", name=f"ue{e}")
            d["t1"] = work.tile([P, F], f16, tag="t1", name=f"t1e{e}")
            d["comb"] = work.tile([P, F], f16, tag="comb", name=f"combe{e}")
            d["g"] = work.tile([P, F], f16, tag="g", name=f"ge{e}")
            d["carryT"] = work.tile([1, P], f16, tag="carryT", padded_shape=[P, P],
                                    name=f"cTe{e}")
            d["p1"] = psum.tile([1, P], f32, tag="p1", name=f"p1e{e}")
            d["pc"] = [psum.tile([P, BANK], f32, tag=f"pc{b}", name=f"pce{e}b{b}")
                       for b in range(nb)]
            st.append(d)

        def stages(e):
            x0, x1 = xs[e]
            q0, q1 = xf[e] if e < NHW else xs[e]
            t = st[e]

            def s1():  # squares (ACT); reads the f32 tiles for HWDGE examples
                nc.scalar.activation(t["sq0"][:], q0[:], AF.Square, bias=0.0, scale=1.0)
                nc.scalar.activation(t["sq1"][:], q1[:], AF.Square, bias=0.0, scale=1.0)

            def s2():  # energy sum (DVE 2x)
                nc.vector.tensor_tensor(t["e"][:], t["sq0"][:], t["sq1"][:], op=OP.add)

            def s3():  # local scans (DVE, fp32 state internally)
                nc.vector.tensor_tensor_scan(
                    t["y"][:], sc(e, 1).broadcast_to([P, F]), t["e"][:], 0.0,
                    op0=OP.mult, op1=OP.add,
                )

            def s4():  # carry matmul 1 + upper-half Ln (overlaps PE chain)
                nc.tensor.matmul(t["p1"][:], t["y"][:, F - 1 : F], mmt(e),
                                 start=True, stop=True)
                if fx < F:
                    nc.scalar.activation(t["d"][:, fx:], t["y"][:, fx:], AF.Ln,
                                         bias=sc(e, 3), scale=sc(e, 2))
                nc.vector.tensor_copy(t["carryT"][:], t["p1"][:])

            def s5():  # carry decay outer product + identity accumulate of y
                for b in range(nb):
                    off = e * nb * BANK
                    nc.tensor.matmul(
                        t["pc"][b][:], t["carryT"][:],
                        dec_t[0:1, off + b * BANK : off + (b + 1) * BANK],
                        start=True, stop=False,
                    )
                    nc.tensor.matmul(
                        t["pc"][b][:], ident, t["y"][:, b * BANK : (b + 1) * BANK],
                        start=False, stop=True,
                    )

            def s6():  # corrected-half Ln straight from PSUM
                for b in range(nb):
                    nc.scalar.activation(t["d"][:, b * BANK : (b + 1) * BANK],
                                         t["pc"][b][:], AF.Ln,
                                         bias=sc(e, 3), scale=sc(e, 2))

            def s7():  # knee clamps (DVE 4x)
                nc.vector.tensor_scalar(t["u"][:], t["d"][:], sc(e, 4), sc(e, 5),
                                        op0=OP.max, op1=OP.min)
                nc.vector.tensor_scalar(t["t1"][:], t["d"][:], sc(e, 5), sc(e, 6),
                                        op0=OP.max, op1=OP.mult)

            def s8():  # knee square (ACT), in place over u (dead after this)
                nc.scalar.activation(t["u"][:], t["u"][:],
                                     AF.Square, bias=sc(e, 8), scale=sc(e, 7))

            def s9():  # combine (DVE 2x)
                nc.vector.tensor_tensor(t["comb"][:], t["t1"][:], t["u"][:], op=OP.add)

            def s10():  # gain (ACT)
                nc.scalar.activation(t["g"][:], t["comb"][:], AF.Exp,
                                     bias=sc(e, 9), scale=-1.0)

            def s11():  # apply gain into the dead square tiles, DMA out w/ cast
                nc.vector.tensor_tensor(t["sq0"][:], t["g"][:], x0[:], op=OP.mult)
                nc.gpsimd.dma_start(out_h[:][e, 0].rearrange("(p i) -> p i", p=P),
                                    t["sq0"][:])
                nc.vector.tensor_tensor(t["sq1"][:], t["g"][:], x1[:], op=OP.mult)
                nc.gpsimd.dma_start(out_h[:][e, 1].rearrange("(p i) -> p i", p=P),
                                    t["sq1"][:])

            return [s1, s2, s3, s4, s5, s6, s7, s8, s9, s10, s11]

        plans = [stages(e) for e in range(NE)]
        nstage = len(plans[0])
        for r in range(nstage + OFF * (NE - 1)):
            for e in range(NE):
                s = r - OFF * e
                if 0 <= s < nstage:
                    plans[e][s]()

    # narrow the ACT table sets so Ln/Exp/Square resolve to the one set that
    # holds all three -> a single table load instead of per-chunk reloads
    import concourse.bacc as bacc_mod

    orig = bacc_mod.get_activation_tables
    strip = {AF.Ln, AF.Exp, AF.Square}

    def patched(arch):
        full = orig(arch)
        return {
            name: (set(fns) if name == "natural_log_exp_and_others"
                   else set(fns) - strip)
            for name, fns in full.items()
        }

    bacc_mod.get_activation_tables = patched
    try:
        nc.compile()
    finally:
        bacc_mod.get_activation_tables = orig
    return nc


def _host_consts(lt, lr, lk, za, nb):
    """Per-core constant tensors from the [NE] parameter vectors (f64 math)."""
    alpha = 1.0 / (1.0 + np.exp(-za))
    thr = lt - 6.0
    r = 1.0 + np.exp(lr)
    c = 1.0 / r - 1.0
    W = np.exp(lk) / 2.0
    s = np.sqrt(-c / (4.0 * W))

    cols = np.zeros((NE, 16))
    cols[:, 1] = alpha
    cols[:, 2] = np.exp(-thr) * (1.0 - alpha) / 2.0   # lnscale
    cols[:, 3] = EPS * np.exp(-thr)                   # lnbias
    cols[:, 4] = -W
    cols[:, 5] = W
    cols[:, 6] = -c
    cols[:, 7] = s
    cols[:, 8] = s * W
    cols[:, 9] = -c * W                               # exp bias
    scal = np.tile(cols.reshape(1, NE * 16), (P, 1)).astype(np.float32)

    # carry matrix, transposed for the matmul: mmt[e][q, p] = A^(p-1-q), q < p
    A = alpha**F
    cmb = np.zeros((P, (NE + 1) * P))
    qs = np.arange(P)
    for e in range(NE):
        for p in range(1, P):
            cmb[:p, e * P + p] = A[e] ** (p - 1 - qs[:p])
    cmb[:, NE * P : (NE + 1) * P] = np.eye(P)
    cmb = cmb.astype(np.float16)

    dec = (alpha[:, None] ** np.arange(1, nb * BANK + 1)[None, :]).astype(np.float16)
    return {"scal": scal, "cmb": cmb, "decay": dec.reshape(1, NE * nb * BANK)}


def _pick_nb(za):
    alpha_max = float(1.0 / (1.0 + np.exp(-np.max(za))))
    alpha_max = min(max(alpha_max, 1e-6), 1.0 - 1e-9)
    need = np.log(1e-10) / np.log(alpha_max)
    return int(min(max(np.ceil(need / BANK), 1), F // BANK))


def _prep(inputs):
    x = np.ascontiguousarray(np.asarray(inputs["input_signals"], np.float32))
    lt = np.asarray(inputs["log_threshold"], np.float64).reshape(N)
    lr = np.asarray(inputs["log_ratio"], np.float64).reshape(N)
    lk = np.asarray(inputs["log_knee"], np.float64).reshape(N)
    za = np.asarray(inputs["z_alpha_pre"], np.float64).reshape(N)
    nb = _pick_nb(za)
    in_maps = []
    for i in range(NCORES):
        s = slice(i * NE, (i + 1) * NE)
        m = {"x": x[s]}
        m.update(_host_consts(lt[s], lr[s], lk[s], za[s], nb))
        in_maps.append(m)
    return nb, in_maps


def _get_nc(nb):
    if nb not in _CACHE:
        _CACHE[nb] = _build(nb)
    return _CACHE[nb]


def _run(inputs, trace=False):
    from concourse.bass_utils import run_bass_kernel_spmd

    nb, in_maps = _prep(inputs)
    nc = _get_nc(nb)
    res = run_bass_kernel_spmd(nc, in_maps, core_ids=list(range(NCORES)), trace=trace)
    out = np.concatenate([res.results[i]["out"] for i in range(NCORES)], axis=0)
    return out, res


def _probe_ok(out, inputs):
    """Recompute the first partition-chunk (no carry needed there) of two
    examples on the host in f64 and compare -- catches a stale compile-cache
    NEFF or a wedged-device garbage execution."""
    x = np.asarray(inputs["input_signals"], np.float64)
    lt = np.asarray(inputs["log_threshold"], np.float64).reshape(N)
    lr = np.asarray(inputs["log_ratio"], np.float64).reshape(N)
    lk = np.asarray(inputs["log_knee"], np.float64).reshape(N)
    za = np.asarray(inputs["z_alpha_pre"], np.float64).reshape(N)
    for e in (0, N - 1):
        a = 1.0 / (1.0 + np.exp(-za[e]))
        en = (1.0 - a) / 2.0 * (x[e, 0, :F] ** 2 + x[e, 1, :F] ** 2)
        y = np.empty(F)
        s = 0.0
        for i in range(F):
            s = a * s + en[i]
            y[i] = s
        d = np.log(y + EPS) - (lt[e] - 6.0)
        r = 1.0 + np.exp(lr[e])
        c = 1.0 / r - 1.0
        W = np.exp(lk[e]) / 2.0
        u = np.clip(d, -W, W)
        q = (u + W) ** 2 / (4.0 * W) + np.maximum(d - W, 0.0)
        g = np.exp(c * q)
        ref = g[None, :] * x[e, :, :F]
        got = out[e, :, :F].astype(np.float64)
        rel = np.linalg.norm(got - ref) / max(np.linalg.norm(ref), 1e-30)
        if not np.isfinite(rel) or rel > 0.02:
            return False
    return True


def kernel(**inputs):
    out = None
    for attempt in range(3):
        out, _ = _run(inputs, trace=False)
        if _probe_ok(out, inputs):
            return out
        # wrong result: drop compiled state (stale NEFF cache / wedged run)
        # and rebuild from scratch
        import os, shutil

        _CACHE.clear()
        cache_dir = os.environ.get(
            "NEURON_COMPILE_CACHE_URL", "/root/.neuron-compile-cache/"
        )
        if cache_dir and os.path.isdir(cache_dir):
            shutil.rmtree(cache_dir, ignore_errors=True)
            os.makedirs(cache_dir, mode=0o700, exist_ok=True)
    return out


# revision 16
# speedup vs baseline: 1.2380x; 1.2380x over previous
"""ApproxCompressor Trainium2 kernel (8 NeuronCores, data parallel over batch).

Algorithm: the reference's FFT convolution with the truncated exponential
impulse response h[n] = (1-a) a^n is a one-pole IIR y[t] = a y[t-1] + (1-a) e[t]
minus a tail term a^16384 y[t-16384] that underflows to zero in float32 for
any alpha = sigmoid(randn).  On-device we therefore run an exact recursive
scan instead of an FFT.

Per core: 4 examples.  Each example's L=131072 samples are laid out as
[128 partitions x 1024]; every DMA is a fully contiguous 512KB HBM transfer
and all 128 partitions scan in parallel (DVE tensor_tensor_scan along the
free dim; the scan state is fp32 internally regardless of output dtype).

The whole elementwise path runs in fp16: x is cast f32->fp16 during the
input DMA (SWDGE cast), which makes every DVE tensor_tensor run in 2x mode
and every tensor_scalar in 4x mode; outputs are cast fp16->f32 during the
output DMA.  The energy scale (1-a)/2 is folded into the Ln scale so the
squares need no scaling.

Cross-chunk scan carries are fixed post-hoc: carry[p] (the true initial
state of chunk p) is linear in the per-chunk final values S, carry = M @ S
with M precomputed on host in f64.  TensorE computes carryT = S^T @ M^T
([1,128]) then pc = carry x decay (rank-1) PLUS an identity-matmul
accumulation of y's first nb*512 columns into the same PSUM tile, so the
corrected envelope for those columns materializes in PSUM with no DVE add;
Ln reads it straight from PSUM (ACT is the engine closest to PSUM).

The quadratic-knee gain is refactored into per-partition-scalar ops:
    d    = ln(lnscale * y + lnbias)            (ACT, scale/bias fold)
    u    = clamp(d, -W, W)                     (DVE tensor_scalar 4x)
    t1   = -c * max(d, W)                      (DVE tensor_scalar 4x)
    sqv  = (s*u + s*W)^2,  s = sqrt(-c/(4W))   (ACT square)
    comb = t1 + sqv                            (DVE tensor_tensor 2x)
    gain = exp(-comb - c*W)                    (ACT, bias fold)
which equals exp(c*q(d)) of the reference knee exactly.  out_c = gain * x_c
is written into the (dead) square tiles and DMA'd out with an fp16->f32 cast.

Emission is software-pipelined: the per-example stage chains are interleaved
with a 2-stage offset so each engine's in-order queue never serializes one
example's tail against the next example's head.
"""

import numpy as np

N, C, L = 32, 2, 131072
NCORES = 8
NE = N // NCORES          # examples per core
P = 128                   # partitions = chunks per example
F = L // P                # 1024 samples per partition
BANK = 512                # psum bank width for the carry fix
EPS = 1e-5
OFF = 1                   # pipeline stage offset between examples

_CACHE = {}


def _build(nb):
    import concourse.bass as bass
    import concourse.tile as tile
    from concourse import bacc, mybir

    f32 = mybir.dt.float32
    f16 = mybir.dt.float16
    AF = mybir.ActivationFunctionType
    OP = mybir.AluOpType

    nc = bacc.Bacc("TRN2", target_bir_lowering=False, debug=False, num_devices=NCORES)

    x_h = nc.declare_dram_parameter("x", [NE, C, L], f32, isOutput=False)
    scal_h = nc.declare_dram_parameter("scal", [P, 16 * NE], f32, isOutput=False)
    cmb_h = nc.declare_dram_parameter("cmb", [P, NE * P], f16, isOutput=False)
    dec_h = nc.declare_dram_parameter("decay", [1, NE * nb * BANK], f16, isOutput=False)
    out_h = nc.declare_dram_parameter("out", [NE, C, L], f32, isOutput=True)

    from contextlib import ExitStack

    with tile.TileContext(nc) as tc, ExitStack() as ctx:
        const = ctx.enter_context(tc.tile_pool(name="const", bufs=1))
        work = ctx.enter_context(tc.tile_pool(name="work", bufs=4))
        xpool = ctx.enter_context(tc.tile_pool(name="xpool", bufs=4))
        psum = ctx.enter_context(tc.tile_pool(name="psum", bufs=2, space="PSUM"))

        # scal leads (its per-partition scalar columns gate every ACT op),
        # then the x casting loads in pipeline order, then carry constants
        # (not needed until the first carry matmul)
        scal_t = const.tile([P, 16 * NE], f32)
        nc.sync.dma_start(scal_t[:], scal_h[:])

        # tiny dummy activation: hoists the ACT table load off the critical
        # path (otherwise it fires only after the first x DMA lands)
        warm_t = const.tile([P, 1], f32)
        nc.scalar.activation(warm_t[:], scal_t[:, 0:1], AF.Exp, bias=0.0, scale=0.0)

        xs = []
        for e in range(NE):
            x0 = xpool.tile([P, F], f16, tag="x0", name=f"x0e{e}")
            x1 = xpool.tile([P, F], f16, tag="x1", name=f"x1e{e}")
            nc.gpsimd.dma_start(x0[:], x_h[:][e, 0].rearrange("(p i) -> p i", p=P))
            nc.gpsimd.dma_start(x1[:], x_h[:][e, 1].rearrange("(p i) -> p i", p=P))
            xs.append((x0, x1))
        cmb_t = const.tile([P, NE * P], f16)
        nc.sync.dma_start(cmb_t[:], cmb_h[:])
        dec_t = const.tile([1, NE * nb * BANK], f16, padded_shape=[P, NE * nb * BANK])
        nc.sync.dma_start(dec_t[:], dec_h[:])

        def sc(e, j):
            return scal_t[:, 16 * e + j : 16 * e + j + 1]

        def mmt(e):
            return cmb_t[:, e * P : (e + 1) * P]



        # per-example tiles, allocated up front so stages can close over them
        st = []
        for e in range(NE):
            d = {}
            d["sq0"] = work.tile([P, F], f16, tag="sq0", name=f"sq0e{e}")
            d["sq1"] = work.tile([P, F], f16, tag="sq1", name=f"sq1e{e}")
            d["e"] = work.tile([P, F], f16, tag="e", name=f"ee{e}")
            d["d"] = work.tile([P, F], f16, tag="d", name=f"de{e}")
            d["u"] = work.tile([P, F], f16, tag="u", name=f"ue{e}")
            d["t1"] = work.tile([P, F], f16, tag="t1", name=f"t1e{e}")
            d["comb"] = work.tile([P, F], f16, tag="comb", name=f"combe{e}")
            d["g"] = work.tile([P, F], f16, tag="g", name=f"ge{e}")
            d["ssb"] = work.tile([P, 1], f16, tag="ssb", name=f"ssbe{e}")
            d["carryT"] = work.tile([1, P], f16, tag="carryT", padded_shape=[P, P],
                                    name=f"cTe{e}")
            d["p1"] = psum.tile([1, P], f32, tag="p1", name=f"p1e{e}")
            # scan output lives in PSUM; the decay matmul accumulates the
            # carry correction straight onto it (start=False)
            d["pc"] = psum.tile([P, F], f32, tag="pc", bufs=3, name=f"pce{e}")
            st.append(d)

        def stages(e):
            x0, x1 = xs[e]
            t = st[e]

            def s1():  # squares (ACT)
                nc.scalar.activation(t["sq0"][:], x0[:], AF.Square, bias=0.0, scale=1.0)
                nc.scalar.activation(t["sq1"][:], x1[:], AF.Square, bias=0.0, scale=1.0)

            def s2():  # energy sum (DVE 2x)
                nc.vector.tensor_tensor(t["e"][:], t["sq0"][:], t["sq1"][:], op=OP.add)

            def s3():  # local scans (DVE, fp32 state) straight into PSUM
                nc.vector.tensor_tensor_scan(
                    t["pc"][:], sc(e, 1).broadcast_to([P, F]), t["e"][:], 0.0,
                    op0=OP.mult, op1=OP.add,
                )

            def s4():  # chunk finals -> SBUF, carry matmul, carryT -> SBUF
                nc.scalar.copy(t["ssb"][:], t["pc"][:, F - 1 : F])
                nc.tensor.matmul(t["p1"][:], t["ssb"][:], mmt(e),
                                 start=True, stop=True)
                nc.vector.tensor_copy(t["carryT"][:], t["p1"][:])

            def s5():  # decay outer product ACCUMULATES onto the scan in PSUM
                for b in range(nb):
                    off = e * nb * BANK
                    nc.tensor.matmul(
                        t["pc"][:, b * BANK : (b + 1) * BANK], t["carryT"][:],
                        dec_t[0:1, off + b * BANK : off + (b + 1) * BANK],
                        start=False, stop=True,
                    )

            def s6():  # full-width Ln straight from PSUM
                nc.scalar.activation(t["d"][:], t["pc"][:], AF.Ln,
                                     bias=sc(e, 3), scale=sc(e, 2))

            def s7():  # knee clamps (DVE 4x)
                nc.vector.tensor_scalar(t["u"][:], t["d"][:], sc(e, 4), sc(e, 5),
                                        op0=OP.max, op1=OP.min)
                nc.vector.tensor_scalar(t["t1"][:], t["d"][:], sc(e, 5), sc(e, 6),
                                        op0=OP.max, op1=OP.mult)

            def s8():  # knee square (ACT), in place over u (dead after this)
                nc.scalar.activation(t["u"][:], t["u"][:],
                                     AF.Square, bias=sc(e, 8), scale=sc(e, 7))

            def s9():  # combine (DVE 2x)
                nc.vector.tensor_tensor(t["comb"][:], t["t1"][:], t["u"][:], op=OP.add)

            def s10():  # gain (ACT)
                nc.scalar.activation(t["g"][:], t["comb"][:], AF.Exp,
                                     bias=sc(e, 9), scale=-1.0)

            def s11():  # apply gain into the dead square tiles, DMA out w/ cast
                nc.vector.tensor_tensor(t["sq0"][:], t["g"][:], x0[:], op=OP.mult)
                nc.gpsimd.dma_start(out_h[:][e, 0].rearrange("(p i) -> p i", p=P),
                                    t["sq0"][:])
                nc.vector.tensor_tensor(t["sq1"][:], t["g"][:], x1[:], op=OP.mult)
                nc.gpsimd.dma_start(out_h[:][e, 1].rearrange("(p i) -> p i", p=P),
                                    t["sq1"][:])

            return [s1, s2, s3, s4, s5, s6, s7, s8, s9, s10, s11]

        plans = [stages(e) for e in range(NE)]
        nstage = len(plans[0])
        for r in range(nstage + OFF * (NE - 1)):
            for e in range(NE):
                s = r - OFF * e
                if 0 <= s < nstage:
                    plans[e][s]()

    # narrow the ACT table sets so Ln/Exp/Square resolve to the one set that
    # holds all three -> a single table load instead of per-chunk reloads
    import concourse.bacc as bacc_mod

    orig = bacc_mod.get_activation_tables
    strip = {AF.Ln, AF.Exp, AF.Square}

    def patched(arch):
        full = orig(arch)
        return {
            name: (set(fns) if name == "natural_log_exp_and_others"
                   else set(fns) - strip)
            for name, fns in full.items()
        }

    bacc_mod.get_activation_tables = patched
    try:
        nc.compile()
    finally:
        bacc_mod.get_activation_tables = orig
    return nc


def _host_consts(lt, lr, lk, za, nb):
    """Per-core constant tensors from the [NE] parameter vectors (f64 math)."""
    alpha = 1.0 / (1.0 + np.exp(-za))
    thr = lt - 6.0
    r = 1.0 + np.exp(lr)
    c = 1.0 / r - 1.0
    W = np.exp(lk) / 2.0
    s = np.sqrt(-c / (4.0 * W))

    cols = np.zeros((NE, 16))
    cols[:, 1] = alpha
    cols[:, 2] = np.exp(-thr) * (1.0 - alpha) / 2.0   # lnscale
    cols[:, 3] = EPS * np.exp(-thr)                   # lnbias
    cols[:, 4] = -W
    cols[:, 5] = W
    cols[:, 6] = -c
    cols[:, 7] = s
    cols[:, 8] = s * W
    cols[:, 9] = -c * W                               # exp bias
    scal = np.tile(cols.reshape(1, NE * 16), (P, 1)).astype(np.float32)

    # carry matrix, transposed for the matmul: mmt[e][q, p] = A^(p-1-q), q < p
    A = alpha**F
    cmb = np.zeros((P, NE * P))
    qs = np.arange(P)
    for e in range(NE):
        for p in range(1, P):
            cmb[:p, e * P + p] = A[e] ** (p - 1 - qs[:p])
    cmb = cmb.astype(np.float16)

    dec = (alpha[:, None] ** np.arange(1, nb * BANK + 1)[None, :]).astype(np.float16)
    return {"scal": scal, "cmb": cmb, "decay": dec.reshape(1, NE * nb * BANK)}


def _pick_nb(za):
    alpha_max = float(1.0 / (1.0 + np.exp(-np.max(za))))
    alpha_max = min(max(alpha_max, 1e-6), 1.0 - 1e-9)
    need = np.log(1e-10) / np.log(alpha_max)
    return int(min(max(np.ceil(need / BANK), 1), F // BANK))


def _prep(inputs):
    x = np.ascontiguousarray(np.asarray(inputs["input_signals"], np.float32))
    lt = np.asarray(inputs["log_threshold"], np.float64).reshape(N)
    lr = np.asarray(inputs["log_ratio"], np.float64).reshape(N)
    lk = np.asarray(inputs["log_knee"], np.float64).reshape(N)
    za = np.asarray(inputs["z_alpha_pre"], np.float64).reshape(N)
    nb = _pick_nb(za)
    in_maps = []
    for i in range(NCORES):
        s = slice(i * NE, (i + 1) * NE)
        m = {"x": x[s]}
        m.update(_host_consts(lt[s], lr[s], lk[s], za[s], nb))
        in_maps.append(m)
    return nb, in_maps


def _get_nc(nb):
    if nb not in _CACHE:
        _CACHE[nb] = _build(nb)
    return _CACHE[nb]


def _run(inputs, trace=False):
    from concourse.bass_utils import run_bass_kernel_spmd

    nb, in_maps = _prep(inputs)
    nc = _get_nc(nb)
    res = run_bass_kernel_spmd(nc, in_maps, core_ids=list(range(NCORES)), trace=trace)
    out = np.concatenate([res.results[i]["out"] for i in range(NCORES)], axis=0)
    return out, res


def _probe_ok(out, inputs):
    """Recompute the first partition-chunk (no carry needed there) of two
    examples on the host in f64 and compare -- catches a stale compile-cache
    NEFF or a wedged-device garbage execution."""
    x = np.asarray(inputs["input_signals"], np.float64)
    lt = np.asarray(inputs["log_threshold"], np.float64).reshape(N)
    lr = np.asarray(inputs["log_ratio"], np.float64).reshape(N)
    lk = np.asarray(inputs["log_knee"], np.float64).reshape(N)
    za = np.asarray(inputs["z_alpha_pre"], np.float64).reshape(N)
    for e in (0, N - 1):
        a = 1.0 / (1.0 + np.exp(-za[e]))
        en = (1.0 - a) / 2.0 * (x[e, 0, :F] ** 2 + x[e, 1, :F] ** 2)
        y = np.empty(F)
        s = 0.0
        for i in range(F):
            s = a * s + en[i]
            y[i] = s
        d = np.log(y + EPS) - (lt[e] - 6.0)
        r = 1.0 + np.exp(lr[e])
        c = 1.0 / r - 1.0
        W = np.exp(lk[e]) / 2.0
        u = np.clip(d, -W, W)
        q = (u + W) ** 2 / (4.0 * W) + np.maximum(d - W, 0.0)
        g = np.exp(c * q)
        ref = g[None, :] * x[e, :, :F]
        got = out[e, :, :F].astype(np.float64)
        rel = np.linalg.norm(got - ref) / max(np.linalg.norm(ref), 1e-30)
        if not np.isfinite(rel) or rel > 0.02:
            return False
    return True


def kernel(**inputs):
    out = None
    for attempt in range(3):
        out, _ = _run(inputs, trace=False)
        if _probe_ok(out, inputs):
            return out
        # wrong result: drop compiled state (stale NEFF cache / wedged run)
        # and rebuild from scratch
        import os, shutil

        _CACHE.clear()
        cache_dir = os.environ.get(
            "NEURON_COMPILE_CACHE_URL", "/root/.neuron-compile-cache/"
        )
        if cache_dir and os.path.isdir(cache_dir):
            shutil.rmtree(cache_dir, ignore_errors=True)
            os.makedirs(cache_dir, mode=0o700, exist_ok=True)
    return out
